# revision 1
# baseline (speedup 1.0000x reference)
"""Trainium2 Bass kernel for nn_DIYloss_1709396984424.

Loss: for binary labels, mean over (one, zero) pairs of (1 + p[l] - p[k])^2
where p = sigmoid(pred_Y). The L^2 pairwise sum collapses to O(L) masked
reductions. With n1 = sum(m), s1 = sum(m*p), s2 = sum(m*p^2), S = sum(p),
T = sum(p^2):

    num   = n1*(T + 2S - 2*s2) + L*(s2 - 2*s1) + 2*s1*(s1 - S)
    denom = max(n1*(L - n1), 1)
    loss  = pair_sum/denom = 1 + num/denom

Each of the 8 cores receives the full (replicated) input and computes the
scalar on-device; core 0's output is returned.

Schedule (engines in parallel, no barriers anywhere):
  SP  : input DMA as the very first instruction, final 4-byte reg store.
  ACT : act-table load (auto), sigmoid, then a Copy-with-accum giving the
        S row sums (copy and sigmoid share one act table set).
  DVE : ones memset, mask cast + n1 row sums (hidden in the wait for the
        sigmoid), three fused product+row-sum ops, PSUM copy, denominator
        chain, scalar epilogue.
  PE  : one [128,5]x[128,1] matmul reducing the partition axis.
  Pool: const-0.0 memset (kept from the framework preamble; the sigmoid
        bias reads it microseconds later), end-of-program DMA-queue +
        semaphore reset (re-execution safety).

Hazard rules (learned the hard way): engine pipelines do NOT interlock
same-engine read-after-write on small operands — an instruction can read
an SBUF cell before the previous instruction's write retires. EVERY RAW
dependency therefore carries a semaphore edge; same-engine edges use the
counting semaphore s_v, where a wait on a later inc covers all earlier
instructions via in-order retirement. Each instruction has at most one
wait (the hardware limit).

The framework's startup barrier and three of its four const-tensor
memsets are suppressed at module-build time (the sigmoid bias uses the
kept const-0.0; nothing references the other three). NEFF executions are
serialized by completion, so the barrier only cost latency. This moves
the DMA trigger from t=666ns to t=0.
"""

import numpy as np

try:
    import concourse.bass as cbass  # noqa: F401
except ImportError:  # pragma: no cover - grading env should have it on path
    import sys

    sys.path.insert(0, "/opt/trn_rl_repo")
    import concourse.bass as cbass  # noqa: F401

from concourse import bacc, mybir
from concourse.bass_utils import run_bass_kernel_spmd

L = 8192
P = 128
F = L // P  # 64
N_CORES = 8

_f32 = mybir.dt.float32
_i32 = mybir.dt.int32
_Alu = mybir.AluOpType
_Act = mybir.ActivationFunctionType
_X = mybir.AxisListType.X

_built = None

# Suppress the framework init preamble (startup all-engine barrier and the
# const memsets other than f32-0.0, which the sigmoid bias uses) while
# constructing the module. The flag is only on during Bacc.__init__.
_suppress = {"on": False}
_orig_memset = cbass.BassSharedVectorInterface.memset
_orig_aeb = cbass.Bass.all_engine_barrier


def _memset_patched(self, ap, constant):
    if _suppress["on"] and constant != 0.0:
        return None
    return _orig_memset(self, ap, constant)


def _aeb_patched(self, *a, **k):
    if _suppress["on"]:
        return None
    return _orig_aeb(self, *a, **k)


cbass.BassSharedVectorInterface.memset = _memset_patched
cbass.Bass.all_engine_barrier = _aeb_patched


def _build():
    _suppress["on"] = True
    try:
        nc = bacc.Bacc(
            "TRN2", debug=False, target_bir_lowering=False, num_devices=N_CORES
        )
    finally:
        _suppress["on"] = False

    # cols 0:F = pred_Y (f32), cols F:2F = true_Y (int32 bitcast to f32)
    xin_d = nc.dram_tensor("xin", [P, 2 * F], _f32, kind="ExternalInput")
    out_d = nc.dram_tensor("out", [1, 1], _f32, kind="ExternalOutput")

    with (
        nc.sbuf_tensor("xt", [P, 2 * F], _f32) as xt,
        nc.sbuf_tensor("p", [P, F], _f32) as p,
        nc.sbuf_tensor("m1", [P, F], _f32) as m1,
        nc.sbuf_tensor("mp", [P, F], _f32) as mp,
        nc.sbuf_tensor("mpp", [P, F], _f32) as mpp,
        nc.sbuf_tensor("p2", [P, F], _f32) as p2,
        nc.sbuf_tensor("sc", [P, F], _f32) as sc,
        nc.sbuf_tensor("stats", [P, 8], _f32) as stats,
        nc.sbuf_tensor("ones", [P, 1], _f32) as ones,
        nc.sbuf_tensor("rw", [1, 32], _f32) as rw,
        nc.psum_tensor("acc", [1, 8], _f32) as acc,
        nc.semaphore("s_in") as s_in,
        nc.semaphore("s_act") as s_act,
        nc.semaphore("s_stats") as s_stats,
        nc.semaphore("s_pe") as s_pe,
        nc.semaphore("s_v") as s_v,
        nc.semaphore("s_done") as s_done,
    ):
        pred_v = xt[:, 0:F]
        true_v = xt[:, F : 2 * F].bitcast(_i32)

        # totals after the PSUM copy: rw[0,0:5] = [colA=T+2S, n1, s1, s2, S]
        T_ = rw[0:1, 0:1]  # holds colA
        n1 = rw[0:1, 1:2]
        s1 = rw[0:1, 2:3]
        s2 = rw[0:1, 3:4]
        S_ = rw[0:1, 4:5]

        def c(i):  # epilogue scratch cells
            return rw[0:1, 8 + i : 9 + i]

        one_c = ones[0:1, 0:1]

        # ---- SP: input DMA first, result store last -------------------
        nc.sync.dma_start(xt[:], xin_d[:]).then_inc(s_in, 16)
        reg = nc.sync.alloc_register()
        nc.sync.reg_load(reg, c(9).bitcast(_i32))._wait_ge(s_v, 10)
        nc.sync.store(out_d[0:1, 0:1].bitcast(_i32), reg).then_inc(s_done, 1)

        # ---- ACT: sigmoid, then Copy-with-accum for S row sums --------
        # bias 0.0 resolves to the kept const-0.0 tile (Pool writes it at
        # t~100ns; the earliest possible sigmoid start is ~2.3us later).
        nc.scalar.activation(p[:], pred_v, _Act.Sigmoid)._wait_ge(
            s_in, 16
        ).then_inc(s_act, 1)
        nc.scalar.activation(
            sc[:], p[:], _Act.Copy, accum_out=stats[:, 4:5]
        )._wait_ge(s_act, 1).then_inc(s_stats, 1)  # own-engine RAW on p

        # ---- DVE: ones, mask cast + n1, fused product sums ------------
        nc.vector.memset(ones[:], 1.0).then_inc(s_stats, 1)
        nc.vector.tensor_copy(m1[:], true_v)._wait_ge(s_in, 16).then_inc(
            s_v, 1
        )  # int -> f32 cast (values 0/1)
        # n1 row sums; hides in the wait for the sigmoid result
        nc.vector.tensor_reduce(
            stats[:, 1:2], m1[:], axis=_X, op=_Alu.add
        )._wait_ge(s_v, 1)
        # stats cols: 0 = colA partials (sum of p^2+2p = T+2S), 1 = n1,
        # 2 = s1, 3 = s2, 4 = S
        # Order mp, tp, mpp: tp depends only on p, so it executes while the
        # mp->mpp same-engine RAW edge (~95ns) resolves.
        nc.vector.scalar_tensor_tensor(
            out=mp[:], in0=m1[:], scalar=1.0, in1=p[:],
            op0=_Alu.mult, op1=_Alu.mult, accum_out=stats[:, 2:3],
        )._wait_ge(s_act, 1).then_inc(s_v, 1)  # 2
        # (p+2)*p = p^2 + 2p; its row sum folds a1 = T+2S into the matmul
        nc.vector.scalar_tensor_tensor(
            out=p2[:], in0=p[:], scalar=2.0, in1=p[:],
            op0=_Alu.add, op1=_Alu.mult, accum_out=stats[:, 0:1],
        )
        nc.vector.scalar_tensor_tensor(
            out=mpp[:], in0=mp[:], scalar=1.0, in1=p[:],
            op0=_Alu.mult, op1=_Alu.mult, accum_out=stats[:, 3:4],
        )._wait_ge(s_v, 2).then_inc(s_stats, 1)  # in-order retire covers tp

        # ---- PE: partition-axis reduction of the 5 stats columns ------
        # s_stats counts three order-independent producers:
        # ones (lhsT), DVE row-sum columns, ACT S column.
        nc.tensor.matmul(
            acc[0:1, 0:5], ones[:], stats[:, 0:5], start=True, stop=True
        )._wait_ge(s_stats, 3).then_inc(s_pe, 1)

        # ---- DVE: totals to SBUF, denominator, scalar epilogue --------
        # Waits are the minimal RAW edges; ops without waits are ordered by
        # the in-order engine and covered transitively by later waits, and
        # pre-dispatch into the exec queue during the matmul wait.
        nc.vector.tensor_copy(rw[0:1, 0:5], acc[0:1, 0:5])._wait_ge(
            s_pe, 1
        ).then_inc(s_v, 1)  # 3
        # d0 = L - n1, reading n1 from PSUM: no semaphore edge needed (the
        # copy's s_pe wait + in-order engine already orders it after the
        # matmul), so it starts the moment the copy's write retires.
        nc.vector.tensor_scalar(
            out=c(16), in0=acc[0:1, 1:2], scalar1=-1.0, scalar2=float(L),
            op0=_Alu.mult, op1=_Alu.add,
        ).then_inc(s_v, 1)  # 4
        # denom = max(n1*d0, 1)
        nc.vector.scalar_tensor_tensor(
            out=c(17), in0=n1, scalar=c(16), in1=one_c,
            op0=_Alu.mult, op1=_Alu.max,
        )._wait_ge(s_v, 4).then_inc(s_v, 1)  # 5
        # r = 1/denom
        nc.vector.reciprocal(c(8), c(17))._wait_ge(s_v, 5)
        # alpha = colA - 2 s2   (colA = T + 2S from the folded column)
        nc.vector.scalar_tensor_tensor(
            out=c(3), in0=s2, scalar=-2.0, in1=T_, op0=_Alu.mult, op1=_Alu.add
        )
        # w = s1 - S
        nc.vector.scalar_tensor_tensor(
            out=c(1), in0=S_, scalar=-1.0, in1=s1, op0=_Alu.mult, op1=_Alu.add
        ).then_inc(s_v, 1)  # 6
        # b1 = s2 - 2 s1
        nc.vector.scalar_tensor_tensor(
            out=c(2), in0=s1, scalar=-2.0, in1=s2, op0=_Alu.mult, op1=_Alu.add
        )
        # q1 = (2 s1) * w
        nc.vector.scalar_tensor_tensor(
            out=c(4), in0=s1, scalar=2.0, in1=c(1), op0=_Alu.mult, op1=_Alu.mult
        )._wait_ge(s_v, 6).then_inc(s_v, 1)  # 7
        # q2 = L*b1 + q1
        nc.vector.scalar_tensor_tensor(
            out=c(5), in0=c(2), scalar=float(L), in1=c(4),
            op0=_Alu.mult, op1=_Alu.add,
        )._wait_ge(s_v, 7).then_inc(s_v, 1)  # 8: b1 covered by in-order retire
        # num = n1*alpha + q2
        nc.vector.scalar_tensor_tensor(
            out=c(6), in0=n1, scalar=c(3), in1=c(5), op0=_Alu.mult, op1=_Alu.add
        )._wait_ge(s_v, 8).then_inc(s_v, 1)  # 9: alpha covered likewise
        # out = num*r + 1
        nc.vector.scalar_tensor_tensor(
            out=c(9), in0=c(6), scalar=c(8), in1=one_c,
            op0=_Alu.mult, op1=_Alu.add,
        )._wait_ge(s_v, 9).then_inc(s_v, 1)  # 10: result ready for SP

        # ---- Pool: self-cleaning tail (no barriers) -------------------
        # s_done (the SP store) transitively implies every semaphore
        # reached its final value and all engines retired their last real
        # instruction. The wait rides ON the drain instruction itself.
        sems = (s_in, s_act, s_stats, s_pe, s_v, s_done)
        sem_lo = min(s.num for s in sems)
        sem_hi = max(s.num for s in sems)
        nc.gpsimd.dma_reset(range(sem_lo, sem_hi + 1))._wait_ge(s_done, 1)
        nc.gpsimd.sem_clear(range(sem_lo, sem_hi + 1))

    nc.compile()
    return nc


def _pack(pred_Y, true_Y):
    xin = np.empty((P, 2 * F), dtype=np.float32)
    xin[:, 0:F] = np.ascontiguousarray(pred_Y, dtype=np.float32).reshape(P, F)
    xin[:, F : 2 * F] = (
        np.ascontiguousarray(true_Y, dtype=np.int32).reshape(P, F).view(np.float32)
    )
    return xin


def _run(pred_Y, true_Y, **hw_kwargs):
    global _built
    if _built is None:
        _built = _build()
    in_map = {"xin": _pack(pred_Y, true_Y)}
    res = run_bass_kernel_spmd(
        _built, [in_map] * N_CORES, list(range(N_CORES)), **hw_kwargs
    )
    out = np.asarray(res.results[0]["out"], dtype=np.float32).reshape(())
    return out, res


def kernel(pred_Y, true_Y):
    out, _ = _run(pred_Y, true_Y)
    return out

